# revision 1
# baseline (speedup 1.0000x reference)
"""Causal multi-head attention block (B=16, S=1024, d=1024, H=16) on 8 NeuronCores.

Strategy: data-parallel over batch (2 batches per core), no collectives.
Per-core kernel (fp16 matmuls, fp32 accumulation):
  phase A: transpose x -> xT[d, m] via PE transposes (cast fp32->fp16)
  phase B: QT = Wq @ xT, KT = Wk @ xT (transposed layout [d_out, m]),
           V  = x @ Wv.T (natural layout [m, d_out], packed in 65-wide
           per-head strips with a fused ones column)
  phase C: per (batch, head, q-chunk): scoresT[k, q] = KT.T @ QT on PE,
           exp((s + mask)/8) on ACT (no max subtraction: |s/8| is O(5)),
           causal mask via 0/1 triangle multiply on diagonal blocks +
           skipping fully-masked blocks, then out_unT[dh, q] (+ sum row,
           from the ones column) = [V|1].T @ expT accumulated on PE.
           Normalize with DVE reciprocal + gpsimd partition_broadcast.
  phase D: y = attn_outT.T @ WoT (natural layout) -> DRAM.
Biases: bq/bk are zero by problem spec (ignored); bv/bo folded in exactly
on the host (y += bv @ Wo.T + bo).
"""

import numpy as np

_CACHE: dict = {}

S = 1024
D = 1024
H = 16
DH = 64
BPC = 2           # batches per core
M = BPC * S       # tokens per core
NCORES = 8


def _build_nc():
    import concourse.bass as bass  # noqa: F401
    import concourse.mybir as mybir
    import concourse.tile as tile
    from concourse import bacc
    from concourse.masks import make_identity
    from contextlib import ExitStack

    f32 = mybir.dt.float32
    f16 = mybir.dt.float16
    EXPF = mybir.ActivationFunctionType.Exp

    nc = bacc.Bacc("TRN2", target_bir_lowering=False, debug=False,
                   num_devices=NCORES)

    x_d = nc.dram_tensor("x", [M, D], f32, kind="ExternalInput")
    wq_d = nc.dram_tensor("Wq", [D, D], f32, kind="ExternalInput")
    wk_d = nc.dram_tensor("Wk", [D, D], f32, kind="ExternalInput")
    wv_d = nc.dram_tensor("Wv", [D, D], f32, kind="ExternalInput")
    wo_d = nc.dram_tensor("Wo", [D, D], f32, kind="ExternalInput")
    tri_d = nc.dram_tensor("tri01", [128, 128], f16, kind="ExternalInput")
    y_d = nc.dram_tensor("y", [M, D], f32, kind="ExternalOutput")

    NMT = M // 128        # 16 m-tiles
    NDT = D // 128        # 8 d-tiles
    NMC = M // 512        # 4 m-chunks
    NOC = D // 512        # 2 o-chunks

    with tile.TileContext(nc) as tc, ExitStack() as top:
        consts = top.enter_context(tc.tile_pool(name="consts", bufs=1))
        persist = top.enter_context(tc.tile_pool(name="persist", bufs=1))
        wrot = top.enter_context(tc.tile_pool(name="wrot", bufs=1))

        ident = consts.tile([128, 128], f32, tag="ident")
        make_identity(nc, ident)
        tri01 = consts.tile([128, 128], f16, tag="tri")
        nc.sync.dma_start(out=tri01, in_=tri_d[:, :])

        # persistent activations (fp16)
        QT = persist.tile([128, NDT, M], f16, tag="QT")    # [o, m] transposed
        KT = persist.tile([128, NDT, M], f16, tag="KT")
        V = persist.tile([128, NMT, H * 65], f16, tag="V")  # [m, head strips]
        AO = persist.tile([128, NDT, M], f16, tag="AO")    # attn out, transposed

        def load_transposed(dst, dram, ncols, stage, psT):
            """dst[:, i_tile, c*128:(c+1)*128] = dram[c*128:(c+1)*128, :].T
            dst: [128, NDT, ncols] fp16; dram: [ncols, D] fp32."""
            for rt in range(ncols // 128):
                st = stage.tile([128, D], f32, tag="stage")
                nc.sync.dma_start(out=st, in_=dram[rt * 128:(rt + 1) * 128, :])
                for g in range(NDT // 4):
                    pt = psT.tile([128, 512], f32, tag="psT")
                    for c in range(4):
                        ib = g * 4 + c
                        nc.tensor.transpose(
                            pt[:, c * 128:(c + 1) * 128],
                            st[:, ib * 128:(ib + 1) * 128], ident)
                    nc.scalar.copy(
                        out=dst[:, g * 4:g * 4 + 4, rt * 128:(rt + 1) * 128],
                        in_=pt.rearrange("p (a b) -> p a b", b=128))

        # ---------- phases A+B: projections ----------
        with ExitStack() as ph1:
            xTp = ph1.enter_context(tc.tile_pool(name="xTp", bufs=1))
            stage = ph1.enter_context(tc.tile_pool(name="stage", bufs=2))
            psT = ph1.enter_context(tc.tile_pool(name="psT", bufs=2, space="PSUM"))
            psP = ph1.enter_context(tc.tile_pool(name="psP", bufs=4, space="PSUM"))

            xT = xTp.tile([128, NDT, M], f16, tag="xT")
            load_transposed(xT, x_d, M, stage, psT)

            # Q and K projections -> transposed layout
            for w_dram, dst in ((wq_d, QT), (wk_d, KT)):
                WT = wrot.tile([128, NDT, D], f16, tag="WT")
                load_transposed(WT, w_dram, D, stage, psT)
                for ot in range(NDT):
                    for mc in range(NMC):
                        pp = psP.tile([128, 512], f32, tag="psP")
                        for it in range(NDT):
                            nc.tensor.matmul(
                                pp,
                                WT[:, it, ot * 128:(ot + 1) * 128],
                                xT[:, it, mc * 512:(mc + 1) * 512],
                                start=(it == 0), stop=(it == NDT - 1))
                        nc.scalar.copy(
                            out=dst[:, ot, mc * 512:(mc + 1) * 512], in_=pp)

            # V projection -> natural layout in 65-wide head strips:
            # every head h: [V(64) | ones] at cols h*65..h*65+64
            WT = wrot.tile([128, NDT, D], f16, tag="WT")
            load_transposed(WT, wv_d, D, stage, psT)
            for mt in range(NMT):
                v2 = V[:, mt, :].rearrange("p (a c) -> p a c", c=65)
                nc.gpsimd.memset(v2[:, :, 64], 1.0)
                for oc in range(NOC):
                    pp = psP.tile([128, 512], f32, tag="psP")
                    for it in range(NDT):
                        nc.tensor.matmul(
                            pp,
                            xT[:, it, mt * 128:(mt + 1) * 128],
                            WT[:, it, oc * 512:(oc + 1) * 512],
                            start=(it == 0), stop=(it == NDT - 1))
                    nc.scalar.copy(
                        out=v2[:, 8 * oc:8 * oc + 8, 0:64],
                        in_=pp.rearrange("p (a c) -> p a c", c=64))

        # ---------- phase C: attention ----------
        LNF = mybir.ActivationFunctionType.Ln
        with ExitStack() as ph2:
            expp = ph2.enter_context(tc.tile_pool(name="expp", bufs=12))
            recp = ph2.enter_context(tc.tile_pool(name="recp", bufs=3))
            rbp = ph2.enter_context(tc.tile_pool(name="rbp", bufs=3))
            tmpp = ph2.enter_context(tc.tile_pool(name="tmpp", bufs=3))
            psS = ph2.enter_context(tc.tile_pool(name="psS", bufs=4, space="PSUM"))
            psO = ph2.enter_context(tc.tile_pool(name="psO", bufs=3, space="PSUM"))

            for b in range(BPC):
                for h in range(H):
                    thq = h // 2
                    po = (h % 2) * 64     # partition offset of this head
                    even = (h % 2 == 0)
                    for qc in range(2):
                        q0 = b * S + qc * 512     # global q start (m coords)
                        ps_o = psO.tile([128, 512], f32, tag="psO")
                        nkt = (qc + 1) * 4
                        for kt in range(nkt):
                            k0 = kt * 128
                            off = max(0, k0 - qc * 512)
                            kg = b * S + k0
                            ps_s = psS.tile([128, 512], f32, tag="psS")
                            nc.tensor.matmul(
                                ps_s[:, off:512],
                                KT[po:po + 64, thq, kg:kg + 128],
                                QT[po:po + 64, thq, q0 + off:q0 + 512],
                                start=True, stop=True)
                            ex = expp.tile([128, 512], f16, tag="exp")
                            nc.scalar.activation(
                                out=ex[:, off:512], in_=ps_s[:, off:512],
                                func=EXPF, scale=0.125)
                            if k0 >= qc * 512:  # diagonal block: 0/1 triangle
                                nc.vector.tensor_mul(
                                    ex[:, off:off + 128],
                                    ex[:, off:off + 128], tri01)
                            mtv = b * (S // 128) + kt
                            nc.tensor.matmul(
                                ps_o[0:65, off:512],
                                V[:, mtv, h * 65:h * 65 + 65],
                                ex[:, off:512],
                                start=(kt == 0), stop=(kt == nkt - 1))
                        # normalize: 1/sums as exp(-ln(sums)) on ACT (the
                        # 1-lane DVE reciprocal costs 3.4us; two ACT LUT
                        # ops are ~3x cheaper and run on a lighter engine)
                        rec = recp.tile([128, 512], f32, tag="rec")
                        nc.scalar.activation(out=rec[64:65, :],
                                             in_=ps_o[64:65, :], func=LNF)
                        nc.scalar.activation(out=rec[64:65, :],
                                             in_=rec[64:65, :], func=EXPF,
                                             scale=-1.0)
                        rb = rbp.tile([64, 512], f32, tag="rb")
                        r1 = rec[64:65, :]
                        rsrc = bass.AP(tensor=r1.tensor, offset=r1.offset,
                                       ap=[list(r1.ap[0]), [0, 64]]
                                       + [list(a) for a in r1.ap[1:]])
                        nc.sync.dma_start(out=rb[0:64, :], in_=rsrc)
                        if even:
                            nc.vector.tensor_mul(
                                out=AO[0:64, thq, q0:q0 + 512],
                                in0=ps_o[0:64, :], in1=rb[0:64, :])
                        else:
                            tmp = tmpp.tile([64, 512], f16, tag="tmp")
                            nc.vector.tensor_mul(
                                out=tmp, in0=ps_o[0:64, :], in1=rb[0:64, :])
                            nc.sync.dma_start(
                                out=AO[64:128, thq, q0:q0 + 512], in_=tmp)

        # ---------- phase D: output projection ----------
        with ExitStack() as ph3:
            stage = ph3.enter_context(tc.tile_pool(name="stage2", bufs=2))
            ystage = ph3.enter_context(tc.tile_pool(name="ystage", bufs=3))
            psT = ph3.enter_context(tc.tile_pool(name="psT2", bufs=2, space="PSUM"))
            psY = ph3.enter_context(tc.tile_pool(name="psY", bufs=4, space="PSUM"))

            WoT = wrot.tile([128, NDT, D], f16, tag="WT")
            load_transposed(WoT, wo_d, D, stage, psT)
            for mt in range(NMT):
                ys = ystage.tile([128, D], f32, tag="ys")
                for oc in range(NOC):
                    pp = psY.tile([128, 512], f32, tag="psY")
                    for dt_ in range(NDT):
                        nc.tensor.matmul(
                            pp,
                            AO[:, dt_, mt * 128:(mt + 1) * 128],
                            WoT[:, dt_, oc * 512:(oc + 1) * 512],
                            start=(dt_ == 0), stop=(dt_ == NDT - 1))
                    nc.scalar.copy(out=ys[:, oc * 512:(oc + 1) * 512], in_=pp)
                nc.sync.dma_start(out=y_d[mt * 128:(mt + 1) * 128, :], in_=ys)

    nc.compile()
    return nc


def _tri01():
    # tri01[dk, dq] = 1 where k <= q (allowed), else 0
    return np.triu(np.ones((128, 128), np.float16))


def _get_nc():
    if "nc" not in _CACHE:
        _CACHE["nc"] = _build_nc()
    return _CACHE["nc"]


def kernel(x, Wq, bq, Wk, bk, Wv, bv, Wo, bo):
    from concourse.bass_utils import run_bass_kernel_spmd

    x = np.ascontiguousarray(np.asarray(x, dtype=np.float32))
    B = x.shape[0]
    assert x.shape == (B, S, D) and B == NCORES * BPC
    Wq = np.ascontiguousarray(np.asarray(Wq, dtype=np.float32))
    Wk = np.ascontiguousarray(np.asarray(Wk, dtype=np.float32))
    Wv = np.ascontiguousarray(np.asarray(Wv, dtype=np.float32))
    Wo = np.ascontiguousarray(np.asarray(Wo, dtype=np.float32))

    nc = _get_nc()
    shards = x.reshape(NCORES, M, D)
    tri = _tri01()
    in_maps = [
        {"x": shards[c], "Wq": Wq, "Wk": Wk, "Wv": Wv, "Wo": Wo, "tri01": tri}
        for c in range(NCORES)
    ]
    res = run_bass_kernel_spmd(nc, in_maps, core_ids=list(range(NCORES)))
    y = np.stack([res.results[c]["y"] for c in range(NCORES)])
    y = y.reshape(B, S, D)

    # exact host-side fold of bv and bo (bq/bk are zero by problem spec)
    bias = (np.asarray(bv, np.float32) @ np.asarray(Wo, np.float32).T
            + np.asarray(bo, np.float32))
    if np.any(bias):
        y = y + bias
    return y.astype(np.float32)



# revision 2
# speedup vs baseline: 1.2955x; 1.2955x over previous
"""Causal multi-head attention block (B=16, S=1024, d=1024, H=16) on 8 NeuronCores.

Strategy: data-parallel over batch (2 batches per core), no collectives.
Host pre-transposes + fp16-casts x and the four weight matrices, so the
device kernel has no transpose phase at all.

Per-core kernel (fp16 matmuls, fp32 accumulation):
  proj(b):  QT/KT = W.T-tiles @ xT (transposed layout [d_out, m]),
            V = x @ Wv.T packed in 65-wide per-head strips with a fused
            ones column (so attn-out matmuls also produce row sums).
  attn(b):  per (head-pair, q-chunk 512): rounds of 2 k-tiles x 2 heads.
            scoresT[k, q] via row-tiled concurrent PE matmuls (head even
            on partitions 0-63, head odd on 64-127), one batched ACT exp
            over a [128, 2048] PSUM strip, causal diag masking via 0/1
            triangle multiplies on DVE, attn-out accumulation on PE.
            Softmax denominators: sum rows gathered to SBUF, batched DVE
            reciprocal per (batch, q-chunk) over all 16 heads, DMA
            partition-broadcast, in-place DVE normalize of AO.
  out(b):   y = AO.T @ WoT per 128-row m-tile, DVE copy, DMA to DRAM.

The attention phase is ACT(exp)-paced, so independent projection /
output-projection matmul "filler" units are interleaved between
attention rounds to keep the PE busy (and the HAM clock warm).
Biases bq/bk are zero by problem spec (ignored); bv/bo folded in
exactly on the host (y += bv @ Wo.T + bo).
"""

import numpy as np

_CACHE: dict = {}

S = 1024
D = 1024
H = 16
DH = 64
BPC = 2           # batches per core
M = BPC * S       # tokens per core
NCORES = 8
NDT = D // 128    # 8 d-tiles


def _build_nc():
    import concourse.bass as bass
    import concourse.mybir as mybir
    import concourse.tile as tile
    from concourse import bacc
    from contextlib import ExitStack
    from collections import deque

    f32 = mybir.dt.float32
    f16 = mybir.dt.float16
    EXPF = mybir.ActivationFunctionType.Exp

    nc = bacc.Bacc("TRN2", target_bir_lowering=False, debug=False,
                   num_devices=NCORES)

    xT_d = nc.dram_tensor("xT", [D, M], f16, kind="ExternalInput")
    wq_d = nc.dram_tensor("WqT", [D, D], f16, kind="ExternalInput")
    wk_d = nc.dram_tensor("WkT", [D, D], f16, kind="ExternalInput")
    wv_d = nc.dram_tensor("WvT", [D, D], f16, kind="ExternalInput")
    wo_d = nc.dram_tensor("WoT", [D, D], f16, kind="ExternalInput")
    tri_d = nc.dram_tensor("tri01", [128, 128], f16, kind="ExternalInput")
    y_d = nc.dram_tensor("y", [M, D], f32, kind="ExternalOutput")

    with tile.TileContext(nc) as tc, ExitStack() as top:
        consts = top.enter_context(tc.tile_pool(name="consts", bufs=1))
        persist = top.enter_context(tc.tile_pool(name="persist", bufs=1))
        wpool = top.enter_context(tc.tile_pool(name="wpool", bufs=2))
        expp = top.enter_context(tc.tile_pool(name="expp", bufs=2))
        sumsp = top.enter_context(tc.tile_pool(name="sumsp", bufs=2))
        stp = top.enter_context(tc.tile_pool(name="stp", bufs=2))
        st32p = top.enter_context(tc.tile_pool(name="st32p", bufs=1))
        rbp = top.enter_context(tc.tile_pool(name="rbp", bufs=2))
        tmpp = top.enter_context(tc.tile_pool(name="tmpp", bufs=2))
        ysp = top.enter_context(tc.tile_pool(name="ysp", bufs=2))
        psS = top.enter_context(tc.tile_pool(name="psS", bufs=1, space="PSUM"))
        psO = top.enter_context(tc.tile_pool(name="psO", bufs=1, space="PSUM"))
        psP = top.enter_context(tc.tile_pool(name="psP", bufs=2, space="PSUM"))

        tri01 = consts.tile([128, 128], f16, tag="tri")
        nc.sync.dma_start(out=tri01, in_=tri_d[:, :])

        # persistent activations (fp16)
        xTs = persist.tile([128, NDT, M], f16, tag="xTs")
        QT = persist.tile([128, NDT, M], f16, tag="QT")    # [d_out, m]
        KT = persist.tile([128, NDT, M], f16, tag="KT")
        V = persist.tile([128, 2 * NDT, H * 65], f16, tag="V")
        AO = persist.tile([128, NDT, M], f16, tag="AO")    # attn out, transposed

        # ones columns of the V strips
        for mt in range(2 * NDT):
            v2 = V[:, mt, :].rearrange("p (a c) -> p a c", c=65)
            nc.gpsimd.memset(v2[:, :, 64], 1.0)

        def load_x(b):
            for dt in range(NDT):
                nc.sync.dma_start(
                    out=xTs[:, dt, b * S:(b + 1) * S],
                    in_=xT_d[dt * 128:(dt + 1) * 128, b * S:(b + 1) * S])

        def load_w_half(w_d, half):
            wt = wpool.tile([128, NDT, 512], f16, tag="W")
            src = w_d[:, half * 512:(half + 1) * 512]
            nc.sync.dma_start(out=wt,
                              in_=src.rearrange("(a p) c -> p a c", p=128))
            return wt

        # ---------- filler units (proj + output proj) ----------
        def proj_qk_units(b, w_d, dst):
            """Q/K projection for batch b: dst[:, ot, m] = W @ x.T."""
            units = []
            state = {}
            for half in range(2):
                def load(half=half, w_d=w_d):
                    state[half] = load_w_half(w_d, half)
                units.append(load)
                for ot_l in range(4):
                    ot = half * 4 + ot_l
                    for mc in (2 * b, 2 * b + 1):
                        def u(half=half, ot=ot, ot_l=ot_l, mc=mc, dst=dst):
                            wt = state[half]
                            pp = psP.tile([128, 512], f32, tag="pp")
                            for it in range(NDT):
                                nc.tensor.matmul(
                                    pp,
                                    wt[:, it, ot_l * 128:(ot_l + 1) * 128],
                                    xTs[:, it, mc * 512:(mc + 1) * 512],
                                    start=(it == 0), stop=(it == NDT - 1))
                            nc.scalar.copy(
                                out=dst[:, ot, mc * 512:(mc + 1) * 512],
                                in_=pp)
                        units.append(u)
            return units

        def proj_v_units(b):
            units = []
            state = {}
            for half in range(2):
                def load(half=half):
                    state[half] = load_w_half(wv_d, half)
                units.append(load)
                for mt_l in range(8):
                    mt = b * 8 + mt_l
                    def u(half=half, mt=mt):
                        wt = state[half]
                        pp = psP.tile([128, 512], f32, tag="pp")
                        for it in range(NDT):
                            nc.tensor.matmul(
                                pp,
                                xTs[:, it, mt * 128:(mt + 1) * 128],
                                wt[:, it, :],
                                start=(it == 0), stop=(it == NDT - 1))
                        v2 = V[:, mt, :].rearrange("p (a c) -> p a c", c=65)
                        nc.vector.tensor_copy(
                            out=v2[:, 8 * half:8 * half + 8, 0:64],
                            in_=pp.rearrange("p (a c) -> p a c", c=64))
                    units.append(u)
            return units

        wo_state = {}

        def wo_load_units():
            def load_a():
                wo_state[0] = load_w_half(wo_d, 0)

            def load_b():
                wo_state[1] = load_w_half(wo_d, 1)
            return [load_a, load_b]

        def d_units(b, qc):
            units = []
            for mt_l in range(4):
                mt = qc * 4 + mt_l
                m0 = b * S + mt * 128
                for oc in range(2):
                    def u(m0=m0, oc=oc):
                        wt = wo_state[oc]
                        pp = psP.tile([128, 512], f32, tag="pp")
                        for dt in range(NDT):
                            nc.tensor.matmul(
                                pp,
                                AO[:, dt, m0:m0 + 128],
                                wt[:, dt, :],
                                start=(dt == 0), stop=(dt == NDT - 1))
                        ys = ysp.tile([128, 512], f32, tag="ys")
                        nc.vector.tensor_copy(out=ys, in_=pp)
                        nc.sync.dma_start(
                            out=y_d[m0:m0 + 128, oc * 512:(oc + 1) * 512],
                            in_=ys)
                    units.append(u)
            return units

        # ---------- attention ----------
        def emit_attn(b, filler):
            def pop_filler(n=1):
                for _ in range(n):
                    if filler:
                        filler.popleft()()

            for qc in range(2):
                st16 = stp.tile([16, 512], f16, tag="st16")
                cols = slice(b * S + qc * 512, b * S + qc * 512 + 512)
                q0 = b * S + qc * 512
                for pair in range(H // 2):
                    nkt = 4 * (qc + 1)
                    pso = psO.tile([128, 1024], f32, tag="pso")
                    for r in range(nkt // 2):
                        strip = psS.tile([128, 2048], f32, tag="strip")
                        # scores: row-tiled concurrent head pairs
                        for ki in range(2):
                            kt = 2 * r + ki
                            kg = b * S + kt * 128
                            for j in range(2):
                                po = j * 64
                                bank = 2 * j + ki
                                nc.tensor.matmul(
                                    strip[:, bank * 512:(bank + 1) * 512],
                                    KT[po:po + 64, pair, kg:kg + 128],
                                    QT[po:po + 64, pair, q0:q0 + 512],
                                    start=True, stop=True)
                        pop_filler()
                        ex = expp.tile([128, 2048], f16, tag="ex")
                        nc.scalar.activation(out=ex, in_=strip, func=EXPF,
                                             scale=0.125)
                        # causal masking on diagonal blocks
                        for ki in range(2):
                            kt = 2 * r + ki
                            off = kt * 128 - qc * 512
                            if off >= 0:
                                for j in range(2):
                                    base = (2 * j + ki) * 512
                                    sl = slice(base + off, base + off + 128)
                                    nc.vector.tensor_mul(ex[:, sl], ex[:, sl],
                                                         tri01)
                        # attn-out accumulation (+ sum row via ones column)
                        for ki in range(2):
                            kt = 2 * r + ki
                            off = max(0, kt * 128 - qc * 512)
                            mtv = b * 8 + kt
                            for j in range(2):
                                h = 2 * pair + j
                                base = (2 * j + ki) * 512
                                nc.tensor.matmul(
                                    pso[0:65, j * 512 + off:j * 512 + 512],
                                    V[:, mtv, h * 65:h * 65 + 65],
                                    ex[:, base + off:base + 512],
                                    start=(kt == 0), stop=(kt == nkt - 1))
                    # unnormalized outputs + sum rows out of PSUM
                    nc.vector.tensor_copy(out=AO[0:64, pair, cols],
                                          in_=pso[0:64, 0:512])
                    tmp = tmpp.tile([64, 512], f16, tag="tmp")
                    nc.vector.tensor_copy(out=tmp, in_=pso[0:64, 512:1024])
                    nc.gpsimd.dma_start(out=AO[64:128, pair, cols], in_=tmp)
                    sums_t = sumsp.tile([128, 1024], f16, tag="sums")
                    nc.scalar.copy(out=sums_t[64:65, :], in_=pso[64:65, :])
                    nc.gpsimd.dma_start(
                        out=st16[2 * pair:2 * pair + 2, :],
                        in_=sums_t[64:65, :].rearrange("p (a c) -> p a c",
                                                       c=512))
                # batched reciprocal over all 16 heads of this (b, qc)
                st32 = st32p.tile([16, 512], f32, tag="st32")
                nc.vector.tensor_copy(out=st32, in_=st16)
                rc32 = st32p.tile([16, 512], f32, tag="rc32")
                nc.vector.reciprocal_approx_fast(out=rc32, in_=st32)
                rc16 = stp.tile([16, 512], f16, tag="rc16")
                nc.vector.tensor_copy(out=rc16, in_=rc32)
                for pair in range(H // 2):
                    rb = rbp.tile([128, 512], f16, tag="rb")
                    r2 = rc16[2 * pair:2 * pair + 2, :]
                    src = bass.AP(tensor=r2.tensor, offset=r2.offset,
                                  ap=[list(r2.ap[0]), [0, 64]]
                                  + [list(a) for a in r2.ap[1:]])
                    nc.gpsimd.dma_start(out=rb, in_=src)
                    nc.vector.tensor_mul(AO[:, pair, cols],
                                         AO[:, pair, cols], rb)
                    pop_filler()
                yield qc

        # ---------- schedule ----------
        filler = deque()
        load_x(0)
        for u in proj_qk_units(0, wq_d, QT):
            u()
        load_x(1)
        for u in proj_qk_units(0, wk_d, KT):
            u()
        for u in proj_v_units(0):
            u()

        filler.extend(proj_qk_units(1, wq_d, QT))
        filler.extend(proj_qk_units(1, wk_d, KT))
        filler.extend(proj_v_units(1))
        filler.extend(wo_load_units())

        for qc in emit_attn(0, filler):
            if qc == 0:
                filler.extend(d_units(0, 0))
        filler.extend(d_units(0, 1))

        for qc in emit_attn(1, filler):
            if qc == 0:
                filler.extend(d_units(1, 0))
        while filler:
            filler.popleft()()
        for u in d_units(1, 1):
            u()

    nc.compile()
    return nc


def _tri01():
    # tri01[dk, dq] = 1 where k <= q (allowed), else 0
    return np.triu(np.ones((128, 128), np.float16))


def _get_nc():
    if "nc" not in _CACHE:
        _CACHE["nc"] = _build_nc()
    return _CACHE["nc"]


def _in_maps(x, Wq, Wk, Wv, Wo):
    """Host-side prep: shard x, transpose + fp16-cast everything."""
    x = np.asarray(x, dtype=np.float32)
    B = x.shape[0]
    assert x.shape == (B, S, D) and B == NCORES * BPC
    shards = x.reshape(NCORES, M, D)
    wqT = np.ascontiguousarray(np.asarray(Wq, np.float32).T.astype(np.float16))
    wkT = np.ascontiguousarray(np.asarray(Wk, np.float32).T.astype(np.float16))
    wvT = np.ascontiguousarray(np.asarray(Wv, np.float32).T.astype(np.float16))
    woT = np.ascontiguousarray(np.asarray(Wo, np.float32).T.astype(np.float16))
    tri = _tri01()
    return [
        {"xT": np.ascontiguousarray(shards[c].T.astype(np.float16)),
         "WqT": wqT, "WkT": wkT, "WvT": wvT, "WoT": woT, "tri01": tri}
        for c in range(NCORES)
    ]


def kernel(x, Wq, bq, Wk, bk, Wv, bv, Wo, bo):
    from concourse.bass_utils import run_bass_kernel_spmd

    nc = _get_nc()
    in_maps = _in_maps(x, Wq, Wk, Wv, Wo)
    res = run_bass_kernel_spmd(nc, in_maps, core_ids=list(range(NCORES)))
    y = np.stack([res.results[c]["y"] for c in range(NCORES)])
    y = y.reshape(NCORES * BPC, S, D)

    # exact host-side fold of bv and bo (bq/bk are zero by problem spec)
    bias = (np.asarray(bv, np.float32) @ np.asarray(Wo, np.float32).T
            + np.asarray(bo, np.float32))
    if np.any(bias):
        y = y + bias
    return y.astype(np.float32)
